# revision 14
# baseline (speedup 1.0000x reference)
"""GNN message-passing layer (out = relu(segment_sum(vals * (xW)[src] by dst)))
on 8 Trainium2 NeuronCores.

Strategy (1D graph partitioning, per sharding hint):
- dst nodes are permuted into 8*BLOCKS blocks of 128, degree-balanced so
  every block has <= C*128 incoming edges; core m owns blocks
  [m*BLOCKS, (m+1)*BLOCKS) and produces those output rows.
- Every core computes the full dense transform h = x @ W (replicated; avoids
  any cross-core communication) into its DRAM, via float32r matmuls.
- Per dst block: one indirect DMA gathers the C*128 source rows h[src] into
  SBUF; a value-scaled one-hot matrix P (built on DVE from iota/is_equal) is
  matmul'd against the messages, accumulating all chunks into one PSUM tile:
  psum[d, f] = sum_e val[e] * [dst_e == d] * h[src_e, f]; relu + store.
- Padding edges carry val = 0 so they contribute nothing.
"""
import math
from contextlib import ExitStack

import numpy as np

import concourse.bass as bass
import concourse.mybir as mybir
import concourse.tile as tile
from concourse.bass_utils import run_bass_kernel_spmd
from concourse.vector_clock import ScopedClock

# --- workaround: this walrus build rejects >1 sync wait per instruction
# ("Too many sync wait commands"). Tile's kernel-tail drain carries one wait
# per active sem lane; give it the same NOP-splitting treatment as everything
# else via a post-schedule legalization pass over all basic blocks. ---
_MAX_WAITS = 1


def _patched_drain_and_barrier(self, tick_clock, wait_clock):
    drain_inst = self.nc.sync.drain()
    wait_clock.add_sem_waits(
        drain_inst.ins, ScopedClock({None: tick_clock.global_clock})
    )
    self.nc.all_engine_barrier()
    popped = self.nc._tile_sem_poison_stack.pop()
    assert popped is self._sem_poison
    self.nc.clear_and_free_semaphores(list(self.sems.allocated().values()))
    self.nc.all_engine_barrier()


tile.TileContext._drain_and_barrier = _patched_drain_and_barrier


def _legalize_sync_waits(nc):
    """Split instructions carrying >_MAX_WAITS sem waits: excess waits move to
    same-engine NOPs inserted immediately before the instruction."""
    n_split = 0
    for f in nc.m.functions:
        for bb in f.blocks:
            out = []
            changed = False
            for ins in bb.instructions:
                si = ins.sync_info
                waits = list(si.on_wait) if si and si.on_wait else []
                if len(waits) > _MAX_WAITS:
                    changed = True
                    n_split += 1
                    for i in range(_MAX_WAITS, len(waits), _MAX_WAITS):
                        nop = mybir.InstNoOp(
                            name=nc.get_next_instruction_name(), ins=[], outs=[]
                        )
                        nop.engine = ins.engine
                        nop.sync_info = mybir.SyncInfo(
                            on_wait=waits[i : i + _MAX_WAITS], on_update=[]
                        )
                        nc.register_instruction(nop, overwrite=True)
                        out.append(nop)
                    si.on_wait = waits[:_MAX_WAITS]
                out.append(ins)
            if changed:
                bb.instructions = out
    return n_split

N_CORES = 8
P = 128


def build_nc(n_pad, d_in, d_out, blocks, C, strip_blocks=8, phase_barrier=False, debug_h=False):
    """One SPMD program: out = relu((A @ x) @ W), gathering x rows directly.
    Gathers depend only on the input x, so the Pool-engine gather chain (the
    bottleneck) starts at t=0 instead of waiting for a dense h = xW pass.
    blocks: dst blocks per core. C: chunks (of 128 edges) per block."""
    f32 = mybir.dt.float32
    f32r = mybir.dt.float32r
    i32 = mybir.dt.int32
    KD = d_in // P

    nc = bass.Bass(num_swdge_queues=4, dynamic_dma_scratch_size=65536)
    xp = nc.declare_dram_parameter("x", [n_pad, d_in], f32r, isOutput=False)
    Wp = nc.declare_dram_parameter("W", [d_in, d_out], f32r, isOutput=False)
    srcp = nc.declare_dram_parameter("src", [P, blocks * C], i32, isOutput=False)
    dstp = nc.declare_dram_parameter("dstv", [P, blocks * C], f32, isOutput=False)
    valp = nc.declare_dram_parameter("valv", [P, blocks * C], f32, isOutput=False)
    idp = nc.declare_dram_parameter("ident", [P, P], f32r, isOutput=False)
    outp = nc.declare_dram_parameter("out", [blocks * P, d_out], f32, isOutput=True)

    with tile.TileContext(nc) as tc:
        with ExitStack() as ctx:
            wpool = ctx.enter_context(tc.tile_pool(name="w", bufs=1))
            epool = ctx.enter_context(tc.tile_pool(name="edges", bufs=1))
            mpool = ctx.enter_context(tc.tile_pool(name="msgs", bufs=3))
            ppool = ctx.enter_context(tc.tile_pool(name="onehot", bufs=2))
            psg = ctx.enter_context(tc.tile_pool(name="psg", bufs=2, space="PSUM"))
            gpool = ctx.enter_context(tc.tile_pool(name="gsb", bufs=2))
            pst = ctx.enter_context(tc.tile_pool(name="pst", bufs=2, space="PSUM"))
            tpool = ctx.enter_context(tc.tile_pool(name="gT", bufs=2))
            pso = ctx.enter_context(tc.tile_pool(name="pso", bufs=2, space="PSUM"))
            opool = ctx.enter_context(tc.tile_pool(name="osb", bufs=2))

            # --- constants / per-core edge data, loaded once ---
            w_t = wpool.tile([P, KD * d_out], f32r)
            for k in range(KD):
                nc.sync.dma_start(
                    w_t[:, k * d_out : (k + 1) * d_out], Wp[k * P : (k + 1) * P, :]
                )
            id_t = wpool.tile([P, P], f32r)
            nc.sync.dma_start(id_t[:], idp[:])
            src_t = epool.tile([P, blocks * C], i32)
            dst_t = epool.tile([P, blocks * C], f32)
            val_t = epool.tile([P, blocks * C], f32)
            nc.sync.dma_start(src_t[:], srcp[:])
            nc.sync.dma_start(dst_t[:], dstp[:])
            nc.sync.dma_start(val_t[:], valp[:])
            iota_t = epool.tile([P, C * P], f32)
            nc.gpsimd.iota(
                iota_t[:],
                pattern=[[0, C], [1, P]],
                base=0,
                channel_multiplier=0,
                allow_small_or_imprecise_dtypes=True,
            )

            # --- per dst block: gather x[src], one-hot scatter matmul to
            # g = A_b @ x, PE-transpose g, then out_b = relu(g @ W) ---
            for b in range(blocks):
                msgs = mpool.tile([P, C * d_in], f32r, tag="msgs")
                # HW indirect DMA honors one offset per partition per
                # instruction -> one gather of 128 rows per chunk.
                for c in range(C):
                    nc.gpsimd.indirect_dma_start(
                        out=msgs[:, c * d_in : (c + 1) * d_in],
                        out_offset=None,
                        in_=xp[:],
                        in_offset=bass.IndirectOffsetOnAxis(
                            ap=src_t[:, b * C + c : b * C + c + 1], axis=0
                        ),
                    )
                pt3 = ppool.tile([P, C * P], f32r, tag="P")
                iota3 = bass.AP(
                    iota_t[:].tensor, iota_t[:].offset,
                    [iota_t[:].ap[0], [P, C], [1, P]],
                )
                p3 = bass.AP(
                    pt3[:].tensor, pt3[:].offset, [pt3[:].ap[0], [P, C], [1, P]]
                )
                dstb = dst_t[:, b * C : (b + 1) * C]
                valb = val_t[:, b * C : (b + 1) * C]
                dst_bc = bass.AP(dstb.tensor, dstb.offset, dstb.ap + [[0, P]])
                val_bc = bass.AP(valb.tensor, valb.offset, valb.ap + [[0, P]])
                nc.vector.tensor_tensor(
                    out=p3, in0=iota3, in1=dst_bc, op=mybir.AluOpType.is_equal
                )
                nc.vector.tensor_tensor(
                    out=p3, in0=p3, in1=val_bc, op=mybir.AluOpType.mult
                )
                # g[dst-slot, k] = sum_e val*x[src_e, k]  -> [128, d_in] PSUM
                g_ps = psg.tile([P, d_in], f32)
                for c in range(C):
                    nc.tensor.matmul(
                        g_ps[:],
                        lhsT=pt3[:, c * P : (c + 1) * P],
                        rhs=msgs[:, c * d_in : (c + 1) * d_in],
                        start=(c == 0),
                        stop=(c == C - 1),
                    )
                g_sb = gpool.tile([P, d_in], f32r, tag="g")
                nc.scalar.activation(
                    g_sb[:], g_ps[:], mybir.ActivationFunctionType.Identity
                )
                # transpose g via identity matmuls, then out_b = relu(gT^T W)
                gT = tpool.tile([P, KD * P], f32r, tag="gT")
                for k in range(KD):
                    t_ps = pst.tile([P, P], f32)
                    nc.tensor.matmul(
                        t_ps[:],
                        lhsT=g_sb[:, k * P : (k + 1) * P],
                        rhs=id_t[:],
                        start=True,
                        stop=True,
                    )
                    nc.vector.tensor_copy(gT[:, k * P : (k + 1) * P], t_ps[:])
                o_ps = pso.tile([P, d_out], f32)
                for k in range(KD):
                    nc.tensor.matmul(
                        o_ps[:],
                        lhsT=gT[:, k * P : (k + 1) * P],
                        rhs=w_t[:, k * d_out : (k + 1) * d_out],
                        start=(k == 0),
                        stop=(k == KD - 1),
                    )
                ot = opool.tile([P, d_out], f32)
                nc.scalar.activation(ot[:], o_ps[:], mybir.ActivationFunctionType.Relu)
                nc.sync.dma_start(outp[b * P : (b + 1) * P, :], ot[:])
    # round-robin indirect gathers across the 4 SWDGE queues (ring-reclaim
    # stalls on a single ring otherwise serialize behind DMA completion)
    qi = 0
    for f in nc.m.functions:
        for bb in f.blocks:
            for ins in bb.instructions:
                if isinstance(ins, mybir.InstDMACopy) and ins.queue == "qPoolDynamic":
                    if qi % 4:
                        ins.queue = f"qPoolDynamic{qi % 4}"
                    qi += 1
    _legalize_sync_waits(nc)
    return nc


def _pack_edges(edge_src, edge_dst, edge_vals, n_nodes, blocks):
    """Permute dst nodes into degree-balanced blocks of 128; pack edges into
    [P, blocks_total*C] per-core arrays (chunk-major columns per block)."""
    import heapq

    total_blocks = N_CORES * blocks
    deg = np.bincount(edge_dst, minlength=n_nodes).astype(np.int64)
    order = np.argsort(-deg, kind="stable")
    heap = [(0, b) for b in range(total_blocks)]
    heapq.heapify(heap)
    counts = np.zeros(total_blocks, np.int32)
    loads = np.zeros(total_blocks, np.int64)
    block_of = np.empty(n_nodes, np.int32)
    slot_of = np.empty(n_nodes, np.int32)
    for node in order:
        while True:
            load, b = heapq.heappop(heap)
            if counts[b] < P:
                break
        block_of[node] = b
        slot_of[node] = counts[b]
        counts[b] += 1
        loads[b] = load + deg[node]
        if counts[b] < P:
            heapq.heappush(heap, (loads[b], b))
    C = max(1, math.ceil(loads.max() / P))

    eb = block_of[edge_dst]
    eorder = np.argsort(eb, kind="stable")
    eb_sorted = eb[eorder]
    bsizes = np.bincount(eb_sorted, minlength=total_blocks)
    boffs = np.concatenate([[0], np.cumsum(bsizes)[:-1]])
    pos = np.arange(len(edge_src)) - boffs[eb_sorted]

    src_a = np.zeros((total_blocks, C, P), np.int32)
    dst_a = np.zeros((total_blocks, C, P), np.float32)
    val_a = np.zeros((total_blocks, C, P), np.float32)
    ch, lane = pos >> 7, pos & 127
    src_a[eb_sorted, ch, lane] = edge_src[eorder]
    dst_a[eb_sorted, ch, lane] = slot_of[edge_dst[eorder]]
    val_a[eb_sorted, ch, lane] = edge_vals[eorder]

    def per_core(a):
        return [
            np.ascontiguousarray(
                a[m * blocks : (m + 1) * blocks].transpose(2, 0, 1).reshape(P, -1)
            )
            for m in range(N_CORES)
        ]

    return per_core(src_a), per_core(dst_a), per_core(val_a), block_of, slot_of, C


def _run(x, W, edge_vals, edge_src, edge_dst, blocks=None, trace=False, phase_barrier=False, trace_cores=None):
    n_nodes, d_in = x.shape
    d_out = W.shape[1]
    if blocks is None:
        blocks = math.ceil(n_nodes / (N_CORES * P))
    n_pad = math.ceil(n_nodes / P) * P
    src_c, dst_c, val_c, block_of, slot_of, C = _pack_edges(
        edge_src, edge_dst, edge_vals, n_nodes, blocks
    )
    xp = np.zeros((n_pad, d_in), np.float32)
    xp[:n_nodes] = x
    ident = np.eye(P, dtype=np.float32)

    nc = build_nc(n_pad, d_in, d_out, blocks, C, phase_barrier=phase_barrier)
    in_maps = [
        {
            "x": xp,
            "W": W,
            "src": src_c[m],
            "dstv": dst_c[m],
            "valv": val_c[m],
            "ident": ident,
        }
        for m in range(N_CORES)
    ]
    res = run_bass_kernel_spmd(
        nc, in_maps, list(range(N_CORES)), trace=trace, trace_cores=trace_cores
    )
    stacked = np.concatenate([res.results[m]["out"] for m in range(N_CORES)], axis=0)
    npc = blocks * P
    gidx = (block_of // blocks) * npc + (block_of % blocks) * P + slot_of
    out = stacked[gidx]
    return out, res


def kernel(x, W, edge_vals, edge_src, edge_dst):
    x = np.asarray(x, np.float32)
    W = np.asarray(W, np.float32)
    edge_vals = np.asarray(edge_vals, np.float32)
    edge_src = np.asarray(edge_src).astype(np.int64)
    edge_dst = np.asarray(edge_dst).astype(np.int64)
    out, _ = _run(x, W, edge_vals, edge_src, edge_dst)
    return out.astype(np.float32)



# revision 16
# speedup vs baseline: 1.2100x; 1.2100x over previous
"""GNN message-passing layer (out = relu(segment_sum(vals * (xW)[src] by dst)))
on 8 Trainium2 NeuronCores.

Strategy (1D graph partitioning, per sharding hint):
- dst nodes are permuted into 8*BLOCKS blocks of 128, degree-balanced so
  every block has <= C*128 incoming edges; core m owns blocks
  [m*BLOCKS, (m+1)*BLOCKS) and produces those output rows.
- Every core computes the full dense transform h = x @ W (replicated; avoids
  any cross-core communication) into its DRAM, via float32r matmuls.
- Per dst block: one indirect DMA gathers the C*128 source rows h[src] into
  SBUF; a value-scaled one-hot matrix P (built on DVE from iota/is_equal) is
  matmul'd against the messages, accumulating all chunks into one PSUM tile:
  psum[d, f] = sum_e val[e] * [dst_e == d] * h[src_e, f]; relu + store.
- Padding edges carry val = 0 so they contribute nothing.
"""
import math
from contextlib import ExitStack

import numpy as np

import concourse.bass as bass
import concourse.mybir as mybir
import concourse.tile as tile
from concourse.bass_utils import run_bass_kernel_spmd
from concourse.vector_clock import ScopedClock

# --- workaround: this walrus build rejects >1 sync wait per instruction
# ("Too many sync wait commands"). Tile's kernel-tail drain carries one wait
# per active sem lane; give it the same NOP-splitting treatment as everything
# else via a post-schedule legalization pass over all basic blocks. ---
_MAX_WAITS = 1


def _patched_drain_and_barrier(self, tick_clock, wait_clock):
    drain_inst = self.nc.sync.drain()
    wait_clock.add_sem_waits(
        drain_inst.ins, ScopedClock({None: tick_clock.global_clock})
    )
    self.nc.all_engine_barrier()
    popped = self.nc._tile_sem_poison_stack.pop()
    assert popped is self._sem_poison
    self.nc.clear_and_free_semaphores(list(self.sems.allocated().values()))
    self.nc.all_engine_barrier()


tile.TileContext._drain_and_barrier = _patched_drain_and_barrier


def _legalize_sync_waits(nc):
    """Split instructions carrying >_MAX_WAITS sem waits: excess waits move to
    same-engine NOPs inserted immediately before the instruction."""
    n_split = 0
    for f in nc.m.functions:
        for bb in f.blocks:
            out = []
            changed = False
            for ins in bb.instructions:
                si = ins.sync_info
                waits = list(si.on_wait) if si and si.on_wait else []
                if len(waits) > _MAX_WAITS:
                    changed = True
                    n_split += 1
                    for i in range(_MAX_WAITS, len(waits), _MAX_WAITS):
                        nop = mybir.InstNoOp(
                            name=nc.get_next_instruction_name(), ins=[], outs=[]
                        )
                        nop.engine = ins.engine
                        nop.sync_info = mybir.SyncInfo(
                            on_wait=waits[i : i + _MAX_WAITS], on_update=[]
                        )
                        nc.register_instruction(nop, overwrite=True)
                        out.append(nop)
                    si.on_wait = waits[:_MAX_WAITS]
                out.append(ins)
            if changed:
                bb.instructions = out
    return n_split

N_CORES = 8
P = 128


def build_nc(n_pad, d_in, d_out, blocks, C, strip_blocks=8, phase_barrier=False, debug_h=False):
    """One SPMD program: out = relu((A @ x) @ W), gathering x rows directly.
    Gathers depend only on the input x, so the Pool-engine gather chain (the
    bottleneck) starts at t=0 instead of waiting for a dense h = xW pass.
    blocks: dst blocks per core. C: chunks (of 128 edges) per block."""
    f32 = mybir.dt.float32
    f32r = mybir.dt.float32r
    i32 = mybir.dt.int32
    KD = d_in // P

    nc = bass.Bass(num_swdge_queues=4)
    xp = nc.declare_dram_parameter("x", [n_pad, d_in], f32r, isOutput=False)
    Wp = nc.declare_dram_parameter("W", [d_in, d_out], f32r, isOutput=False)
    srcp = nc.declare_dram_parameter("src", [P, blocks * C], i32, isOutput=False)
    dstp = nc.declare_dram_parameter("dstv", [P, blocks * C], f32, isOutput=False)
    valp = nc.declare_dram_parameter("valv", [P, blocks * C], f32, isOutput=False)
    idp = nc.declare_dram_parameter("ident", [P, P], f32r, isOutput=False)
    outp = nc.declare_dram_parameter("out", [blocks * P, d_out], f32, isOutput=True)

    with tile.TileContext(nc) as tc:
        with ExitStack() as ctx:
            wpool = ctx.enter_context(tc.tile_pool(name="w", bufs=1))
            epool = ctx.enter_context(tc.tile_pool(name="edges", bufs=1))
            mpool = ctx.enter_context(tc.tile_pool(name="msgs", bufs=2))
            ppool = ctx.enter_context(tc.tile_pool(name="onehot", bufs=2))
            psg = ctx.enter_context(tc.tile_pool(name="psg", bufs=2, space="PSUM"))
            gpool = ctx.enter_context(tc.tile_pool(name="gsb", bufs=2))
            pst = ctx.enter_context(tc.tile_pool(name="pst", bufs=2, space="PSUM"))
            tpool = ctx.enter_context(tc.tile_pool(name="gT", bufs=2))
            pso = ctx.enter_context(tc.tile_pool(name="pso", bufs=2, space="PSUM"))
            opool = ctx.enter_context(tc.tile_pool(name="osb", bufs=2))

            # --- constants / per-core edge data, loaded once ---
            w_t = wpool.tile([P, KD * d_out], f32r)
            for k in range(KD):
                nc.sync.dma_start(
                    w_t[:, k * d_out : (k + 1) * d_out], Wp[k * P : (k + 1) * P, :]
                )
            id_t = wpool.tile([P, P], f32r)
            nc.sync.dma_start(id_t[:], idp[:])
            src_t = epool.tile([P, blocks * C], i32)
            dst_t = epool.tile([P, blocks * C], f32)
            val_t = epool.tile([P, blocks * C], f32)
            nc.sync.dma_start(src_t[:], srcp[:])
            nc.sync.dma_start(dst_t[:], dstp[:])
            nc.sync.dma_start(val_t[:], valp[:])
            iota_t = epool.tile([P, C * P], f32)
            nc.gpsimd.iota(
                iota_t[:],
                pattern=[[0, C], [1, P]],
                base=0,
                channel_multiplier=0,
                allow_small_or_imprecise_dtypes=True,
            )

            # --- per dst block: gather x[src], one-hot scatter matmul to
            # g = A_b @ x, PE-transpose g, then out_b = relu(g @ W) ---
            for b in range(blocks):
                msgs = mpool.tile([P, C * d_in], f32r, tag="msgs")
                # HW indirect DMA honors one offset per partition per
                # instruction -> one gather of 128 rows per chunk.
                for c in range(C):
                    nc.gpsimd.indirect_dma_start(
                        out=msgs[:, c * d_in : (c + 1) * d_in],
                        out_offset=None,
                        in_=xp[:],
                        in_offset=bass.IndirectOffsetOnAxis(
                            ap=src_t[:, b * C + c : b * C + c + 1], axis=0
                        ),
                    )
                pt3 = ppool.tile([P, C * P], f32r, tag="P")
                iota3 = bass.AP(
                    iota_t[:].tensor, iota_t[:].offset,
                    [iota_t[:].ap[0], [P, C], [1, P]],
                )
                p3 = bass.AP(
                    pt3[:].tensor, pt3[:].offset, [pt3[:].ap[0], [P, C], [1, P]]
                )
                dstb = dst_t[:, b * C : (b + 1) * C]
                valb = val_t[:, b * C : (b + 1) * C]
                dst_bc = bass.AP(dstb.tensor, dstb.offset, dstb.ap + [[0, P]])
                val_bc = bass.AP(valb.tensor, valb.offset, valb.ap + [[0, P]])
                nc.vector.tensor_tensor(
                    out=p3, in0=iota3, in1=dst_bc, op=mybir.AluOpType.is_equal
                )
                nc.vector.tensor_tensor(
                    out=p3, in0=p3, in1=val_bc, op=mybir.AluOpType.mult
                )
                # g[dst-slot, k] = sum_e val*x[src_e, k]  -> [128, d_in] PSUM
                g_ps = psg.tile([P, d_in], f32)
                for c in range(C):
                    nc.tensor.matmul(
                        g_ps[:],
                        lhsT=pt3[:, c * P : (c + 1) * P],
                        rhs=msgs[:, c * d_in : (c + 1) * d_in],
                        start=(c == 0),
                        stop=(c == C - 1),
                    )
                g_sb = gpool.tile([P, d_in], f32r, tag="g")
                nc.scalar.activation(
                    g_sb[:], g_ps[:], mybir.ActivationFunctionType.Identity
                )
                # transpose g via identity matmuls, then out_b = relu(gT^T W)
                gT = tpool.tile([P, KD * P], f32r, tag="gT")
                for k in range(KD):
                    t_ps = pst.tile([P, P], f32)
                    nc.tensor.matmul(
                        t_ps[:],
                        lhsT=g_sb[:, k * P : (k + 1) * P],
                        rhs=id_t[:],
                        start=True,
                        stop=True,
                    )
                    nc.vector.tensor_copy(gT[:, k * P : (k + 1) * P], t_ps[:])
                o_ps = pso.tile([P, d_out], f32)
                for k in range(KD):
                    nc.tensor.matmul(
                        o_ps[:],
                        lhsT=gT[:, k * P : (k + 1) * P],
                        rhs=w_t[:, k * d_out : (k + 1) * d_out],
                        start=(k == 0),
                        stop=(k == KD - 1),
                    )
                ot = opool.tile([P, d_out], f32)
                nc.scalar.activation(ot[:], o_ps[:], mybir.ActivationFunctionType.Relu)
                nc.sync.dma_start(outp[b * P : (b + 1) * P, :], ot[:])
    # round-robin indirect gathers across the 4 SWDGE queues (ring-reclaim
    # stalls on a single ring otherwise serialize behind DMA completion)
    qi = 0
    for f in nc.m.functions:
        for bb in f.blocks:
            for ins in bb.instructions:
                if isinstance(ins, mybir.InstDMACopy) and ins.queue == "qPoolDynamic":
                    if qi % 4:
                        ins.queue = f"qPoolDynamic{qi % 4}"
                    qi += 1
    _legalize_sync_waits(nc)
    return nc


def _pack_edges(edge_src, edge_dst, edge_vals, n_nodes, blocks):
    """Permute dst nodes into degree-balanced blocks of 128; pack edges into
    [P, blocks_total*C] per-core arrays (chunk-major columns per block)."""
    import heapq

    total_blocks = N_CORES * blocks
    deg = np.bincount(edge_dst, minlength=n_nodes).astype(np.int64)
    order = np.argsort(-deg, kind="stable")
    heap = [(0, b) for b in range(total_blocks)]
    heapq.heapify(heap)
    counts = np.zeros(total_blocks, np.int32)
    loads = np.zeros(total_blocks, np.int64)
    block_of = np.empty(n_nodes, np.int32)
    slot_of = np.empty(n_nodes, np.int32)
    for node in order:
        while True:
            load, b = heapq.heappop(heap)
            if counts[b] < P:
                break
        block_of[node] = b
        slot_of[node] = counts[b]
        counts[b] += 1
        loads[b] = load + deg[node]
        if counts[b] < P:
            heapq.heappush(heap, (loads[b], b))
    C = max(1, math.ceil(loads.max() / P))

    eb = block_of[edge_dst]
    eorder = np.argsort(eb, kind="stable")
    eb_sorted = eb[eorder]
    bsizes = np.bincount(eb_sorted, minlength=total_blocks)
    boffs = np.concatenate([[0], np.cumsum(bsizes)[:-1]])
    pos = np.arange(len(edge_src)) - boffs[eb_sorted]

    src_a = np.zeros((total_blocks, C, P), np.int32)
    dst_a = np.zeros((total_blocks, C, P), np.float32)
    val_a = np.zeros((total_blocks, C, P), np.float32)
    ch, lane = pos >> 7, pos & 127
    src_a[eb_sorted, ch, lane] = edge_src[eorder]
    dst_a[eb_sorted, ch, lane] = slot_of[edge_dst[eorder]]
    val_a[eb_sorted, ch, lane] = edge_vals[eorder]

    def per_core(a):
        return [
            np.ascontiguousarray(
                a[m * blocks : (m + 1) * blocks].transpose(2, 0, 1).reshape(P, -1)
            )
            for m in range(N_CORES)
        ]

    return per_core(src_a), per_core(dst_a), per_core(val_a), block_of, slot_of, C


def _run(x, W, edge_vals, edge_src, edge_dst, blocks=None, trace=False, phase_barrier=False, trace_cores=None):
    n_nodes, d_in = x.shape
    d_out = W.shape[1]
    if blocks is None:
        blocks = math.ceil(n_nodes / (N_CORES * P))
    n_pad = math.ceil(n_nodes / P) * P
    src_c, dst_c, val_c, block_of, slot_of, C = _pack_edges(
        edge_src, edge_dst, edge_vals, n_nodes, blocks
    )
    xp = np.zeros((n_pad, d_in), np.float32)
    xp[:n_nodes] = x
    ident = np.eye(P, dtype=np.float32)

    nc = build_nc(n_pad, d_in, d_out, blocks, C, phase_barrier=phase_barrier)
    in_maps = [
        {
            "x": xp,
            "W": W,
            "src": src_c[m],
            "dstv": dst_c[m],
            "valv": val_c[m],
            "ident": ident,
        }
        for m in range(N_CORES)
    ]
    res = run_bass_kernel_spmd(
        nc, in_maps, list(range(N_CORES)), trace=trace, trace_cores=trace_cores
    )
    stacked = np.concatenate([res.results[m]["out"] for m in range(N_CORES)], axis=0)
    npc = blocks * P
    gidx = (block_of // blocks) * npc + (block_of % blocks) * P + slot_of
    out = stacked[gidx]
    return out, res


def kernel(x, W, edge_vals, edge_src, edge_dst):
    x = np.asarray(x, np.float32)
    W = np.asarray(W, np.float32)
    edge_vals = np.asarray(edge_vals, np.float32)
    edge_src = np.asarray(edge_src).astype(np.int64)
    edge_dst = np.asarray(edge_dst).astype(np.int64)
    out, _ = _run(x, W, edge_vals, edge_src, edge_dst)
    return out.astype(np.float32)



# revision 17
# speedup vs baseline: 1.2132x; 1.0027x over previous
"""GNN message-passing layer (out = relu(segment_sum(vals * (xW)[src] by dst)))
on 8 Trainium2 NeuronCores.

Strategy (1D graph partitioning, per sharding hint):
- dst nodes are permuted into 8*BLOCKS blocks of 128, degree-balanced so
  every block has <= C*128 incoming edges; core m owns blocks
  [m*BLOCKS, (m+1)*BLOCKS) and produces those output rows.
- Every core computes the full dense transform h = x @ W (replicated; avoids
  any cross-core communication) into its DRAM, via float32r matmuls.
- Per dst block: one indirect DMA gathers the C*128 source rows h[src] into
  SBUF; a value-scaled one-hot matrix P (built on DVE from iota/is_equal) is
  matmul'd against the messages, accumulating all chunks into one PSUM tile:
  psum[d, f] = sum_e val[e] * [dst_e == d] * h[src_e, f]; relu + store.
- Padding edges carry val = 0 so they contribute nothing.
"""
import math
from contextlib import ExitStack

import numpy as np

import concourse.bass as bass
import concourse.mybir as mybir
import concourse.tile as tile
from concourse.bass_utils import run_bass_kernel_spmd
from concourse.vector_clock import ScopedClock

# --- workaround: this walrus build rejects >1 sync wait per instruction
# ("Too many sync wait commands"). Tile's kernel-tail drain carries one wait
# per active sem lane; give it the same NOP-splitting treatment as everything
# else via a post-schedule legalization pass over all basic blocks. ---
_MAX_WAITS = 1


def _patched_drain_and_barrier(self, tick_clock, wait_clock):
    drain_inst = self.nc.sync.drain()
    wait_clock.add_sem_waits(
        drain_inst.ins, ScopedClock({None: tick_clock.global_clock})
    )
    self.nc.all_engine_barrier()
    popped = self.nc._tile_sem_poison_stack.pop()
    assert popped is self._sem_poison
    self.nc.clear_and_free_semaphores(list(self.sems.allocated().values()))
    self.nc.all_engine_barrier()


tile.TileContext._drain_and_barrier = _patched_drain_and_barrier


def _legalize_sync_waits(nc):
    """Split instructions carrying >_MAX_WAITS sem waits: excess waits move to
    same-engine NOPs inserted immediately before the instruction."""
    n_split = 0
    for f in nc.m.functions:
        for bb in f.blocks:
            out = []
            changed = False
            for ins in bb.instructions:
                si = ins.sync_info
                waits = list(si.on_wait) if si and si.on_wait else []
                if len(waits) > _MAX_WAITS:
                    changed = True
                    n_split += 1
                    for i in range(_MAX_WAITS, len(waits), _MAX_WAITS):
                        nop = mybir.InstNoOp(
                            name=nc.get_next_instruction_name(), ins=[], outs=[]
                        )
                        nop.engine = ins.engine
                        nop.sync_info = mybir.SyncInfo(
                            on_wait=waits[i : i + _MAX_WAITS], on_update=[]
                        )
                        nc.register_instruction(nop, overwrite=True)
                        out.append(nop)
                    si.on_wait = waits[:_MAX_WAITS]
                out.append(ins)
            if changed:
                bb.instructions = out
    return n_split

N_CORES = 8
P = 128


def build_nc(n_pad, d_in, d_out, blocks, C, strip_blocks=8, phase_barrier=False, debug_h=False):
    """One SPMD program: out = relu((A @ x) @ W), gathering x rows directly.
    Gathers depend only on the input x, so the Pool-engine gather chain (the
    bottleneck) starts at t=0 instead of waiting for a dense h = xW pass.
    blocks: dst blocks per core. C: chunks (of 128 edges) per block."""
    f32 = mybir.dt.float32
    f32r = mybir.dt.float32r
    i32 = mybir.dt.int32
    KD = d_in // P

    nc = bass.Bass(num_swdge_queues=4)
    xp = nc.declare_dram_parameter("x", [n_pad, d_in], f32r, isOutput=False)
    Wp = nc.declare_dram_parameter("W", [d_in, d_out], f32r, isOutput=False)
    srcp = nc.declare_dram_parameter("src", [P, blocks * C], i32, isOutput=False)
    dstp = nc.declare_dram_parameter("dstv", [P, blocks * C], f32, isOutput=False)
    valp = nc.declare_dram_parameter("valv", [P, blocks * C], f32, isOutput=False)
    idp = nc.declare_dram_parameter("ident", [P, P], f32r, isOutput=False)
    outp = nc.declare_dram_parameter("out", [blocks * P, d_out], f32, isOutput=True)

    with tile.TileContext(nc) as tc:
        with ExitStack() as ctx:
            wpool = ctx.enter_context(tc.tile_pool(name="w", bufs=1))
            epool = ctx.enter_context(tc.tile_pool(name="edges", bufs=1))
            mpool = ctx.enter_context(tc.tile_pool(name="msgs", bufs=3))
            ppool = ctx.enter_context(tc.tile_pool(name="onehot", bufs=2))
            psg = ctx.enter_context(tc.tile_pool(name="psg", bufs=2, space="PSUM"))
            gpool = ctx.enter_context(tc.tile_pool(name="gsb", bufs=2))
            pst = ctx.enter_context(tc.tile_pool(name="pst", bufs=2, space="PSUM"))
            tpool = ctx.enter_context(tc.tile_pool(name="gT", bufs=2))
            pso = ctx.enter_context(tc.tile_pool(name="pso", bufs=2, space="PSUM"))
            opool = ctx.enter_context(tc.tile_pool(name="osb", bufs=2))

            # --- constants / per-core edge data, loaded once ---
            w_t = wpool.tile([P, KD * d_out], f32r)
            for k in range(KD):
                nc.sync.dma_start(
                    w_t[:, k * d_out : (k + 1) * d_out], Wp[k * P : (k + 1) * P, :]
                )
            id_t = wpool.tile([P, P], f32r)
            nc.sync.dma_start(id_t[:], idp[:])
            src_t = epool.tile([P, blocks * C], i32)
            dst_t = epool.tile([P, blocks * C], f32)
            val_t = epool.tile([P, blocks * C], f32)
            nc.sync.dma_start(src_t[:], srcp[:])
            nc.sync.dma_start(dst_t[:], dstp[:])
            nc.sync.dma_start(val_t[:], valp[:])
            iota_t = epool.tile([P, C * P], f32)
            nc.gpsimd.iota(
                iota_t[:],
                pattern=[[0, C], [1, P]],
                base=0,
                channel_multiplier=0,
                allow_small_or_imprecise_dtypes=True,
            )

            # --- per dst block: gather x[src], one-hot scatter matmul to
            # g = A_b @ x, PE-transpose g, then out_b = relu(g @ W) ---
            for b in range(blocks):
                msgs = mpool.tile([P, C * d_in], f32r, tag="msgs")
                # HW indirect DMA honors one offset per partition per
                # instruction -> one gather of 128 rows per chunk.
                for c in range(C):
                    nc.gpsimd.indirect_dma_start(
                        out=msgs[:, c * d_in : (c + 1) * d_in],
                        out_offset=None,
                        in_=xp[:],
                        in_offset=bass.IndirectOffsetOnAxis(
                            ap=src_t[:, b * C + c : b * C + c + 1], axis=0
                        ),
                    )
                pt3 = ppool.tile([P, C * P], f32r, tag="P")
                iota3 = bass.AP(
                    iota_t[:].tensor, iota_t[:].offset,
                    [iota_t[:].ap[0], [P, C], [1, P]],
                )
                p3 = bass.AP(
                    pt3[:].tensor, pt3[:].offset, [pt3[:].ap[0], [P, C], [1, P]]
                )
                dstb = dst_t[:, b * C : (b + 1) * C]
                valb = val_t[:, b * C : (b + 1) * C]
                dst_bc = bass.AP(dstb.tensor, dstb.offset, dstb.ap + [[0, P]])
                val_bc = bass.AP(valb.tensor, valb.offset, valb.ap + [[0, P]])
                nc.vector.tensor_tensor(
                    out=p3, in0=iota3, in1=dst_bc, op=mybir.AluOpType.is_equal
                )
                nc.vector.tensor_tensor(
                    out=p3, in0=p3, in1=val_bc, op=mybir.AluOpType.mult
                )
                # g[dst-slot, k] = sum_e val*x[src_e, k]  -> [128, d_in] PSUM
                g_ps = psg.tile([P, d_in], f32)
                for c in range(C):
                    nc.tensor.matmul(
                        g_ps[:],
                        lhsT=pt3[:, c * P : (c + 1) * P],
                        rhs=msgs[:, c * d_in : (c + 1) * d_in],
                        start=(c == 0),
                        stop=(c == C - 1),
                    )
                g_sb = gpool.tile([P, d_in], f32r, tag="g")
                nc.scalar.activation(
                    g_sb[:], g_ps[:], mybir.ActivationFunctionType.Identity
                )
                # transpose g via identity matmuls, then out_b = relu(gT^T W)
                gT = tpool.tile([P, KD * P], f32r, tag="gT")
                for k in range(KD):
                    t_ps = pst.tile([P, P], f32)
                    nc.tensor.matmul(
                        t_ps[:],
                        lhsT=g_sb[:, k * P : (k + 1) * P],
                        rhs=id_t[:],
                        start=True,
                        stop=True,
                    )
                    nc.vector.tensor_copy(gT[:, k * P : (k + 1) * P], t_ps[:])
                o_ps = pso.tile([P, d_out], f32)
                for k in range(KD):
                    nc.tensor.matmul(
                        o_ps[:],
                        lhsT=gT[:, k * P : (k + 1) * P],
                        rhs=w_t[:, k * d_out : (k + 1) * d_out],
                        start=(k == 0),
                        stop=(k == KD - 1),
                    )
                ot = opool.tile([P, d_out], f32)
                nc.scalar.activation(ot[:], o_ps[:], mybir.ActivationFunctionType.Relu)
                nc.sync.dma_start(outp[b * P : (b + 1) * P, :], ot[:])
    # round-robin indirect gathers across the 4 SWDGE queues (ring-reclaim
    # stalls on a single ring otherwise serialize behind DMA completion)
    qi = 0
    for f in nc.m.functions:
        for bb in f.blocks:
            for ins in bb.instructions:
                if isinstance(ins, mybir.InstDMACopy) and ins.queue == "qPoolDynamic":
                    if qi % 4:
                        ins.queue = f"qPoolDynamic{qi % 4}"
                    qi += 1
    _legalize_sync_waits(nc)
    return nc


def _pack_edges(edge_src, edge_dst, edge_vals, n_nodes, blocks):
    """Permute dst nodes into degree-balanced blocks of 128; pack edges into
    [P, blocks_total*C] per-core arrays (chunk-major columns per block)."""
    import heapq

    total_blocks = N_CORES * blocks
    deg = np.bincount(edge_dst, minlength=n_nodes).astype(np.int64)
    order = np.argsort(-deg, kind="stable")
    heap = [(0, b) for b in range(total_blocks)]
    heapq.heapify(heap)
    counts = np.zeros(total_blocks, np.int32)
    loads = np.zeros(total_blocks, np.int64)
    block_of = np.empty(n_nodes, np.int32)
    slot_of = np.empty(n_nodes, np.int32)
    for node in order:
        while True:
            load, b = heapq.heappop(heap)
            if counts[b] < P:
                break
        block_of[node] = b
        slot_of[node] = counts[b]
        counts[b] += 1
        loads[b] = load + deg[node]
        if counts[b] < P:
            heapq.heappush(heap, (loads[b], b))
    C = max(1, math.ceil(loads.max() / P))

    eb = block_of[edge_dst]
    eorder = np.argsort(eb, kind="stable")
    eb_sorted = eb[eorder]
    bsizes = np.bincount(eb_sorted, minlength=total_blocks)
    boffs = np.concatenate([[0], np.cumsum(bsizes)[:-1]])
    pos = np.arange(len(edge_src)) - boffs[eb_sorted]

    src_a = np.zeros((total_blocks, C, P), np.int32)
    dst_a = np.zeros((total_blocks, C, P), np.float32)
    val_a = np.zeros((total_blocks, C, P), np.float32)
    ch, lane = pos >> 7, pos & 127
    src_a[eb_sorted, ch, lane] = edge_src[eorder]
    dst_a[eb_sorted, ch, lane] = slot_of[edge_dst[eorder]]
    val_a[eb_sorted, ch, lane] = edge_vals[eorder]

    def per_core(a):
        return [
            np.ascontiguousarray(
                a[m * blocks : (m + 1) * blocks].transpose(2, 0, 1).reshape(P, -1)
            )
            for m in range(N_CORES)
        ]

    return per_core(src_a), per_core(dst_a), per_core(val_a), block_of, slot_of, C


def _run(x, W, edge_vals, edge_src, edge_dst, blocks=None, trace=False, phase_barrier=False, trace_cores=None):
    n_nodes, d_in = x.shape
    d_out = W.shape[1]
    if blocks is None:
        blocks = math.ceil(n_nodes / (N_CORES * P))
    n_pad = math.ceil(n_nodes / P) * P
    src_c, dst_c, val_c, block_of, slot_of, C = _pack_edges(
        edge_src, edge_dst, edge_vals, n_nodes, blocks
    )
    xp = np.zeros((n_pad, d_in), np.float32)
    xp[:n_nodes] = x
    ident = np.eye(P, dtype=np.float32)

    nc = build_nc(n_pad, d_in, d_out, blocks, C, phase_barrier=phase_barrier)
    in_maps = [
        {
            "x": xp,
            "W": W,
            "src": src_c[m],
            "dstv": dst_c[m],
            "valv": val_c[m],
            "ident": ident,
        }
        for m in range(N_CORES)
    ]
    res = run_bass_kernel_spmd(
        nc, in_maps, list(range(N_CORES)), trace=trace, trace_cores=trace_cores
    )
    stacked = np.concatenate([res.results[m]["out"] for m in range(N_CORES)], axis=0)
    npc = blocks * P
    gidx = (block_of // blocks) * npc + (block_of % blocks) * P + slot_of
    out = stacked[gidx]
    return out, res


def kernel(x, W, edge_vals, edge_src, edge_dst):
    x = np.asarray(x, np.float32)
    W = np.asarray(W, np.float32)
    edge_vals = np.asarray(edge_vals, np.float32)
    edge_src = np.asarray(edge_src).astype(np.int64)
    edge_dst = np.asarray(edge_dst).astype(np.int64)
    out, _ = _run(x, W, edge_vals, edge_src, edge_dst)
    return out.astype(np.float32)



# revision 19
# speedup vs baseline: 1.2172x; 1.0033x over previous
"""GNN message-passing layer (out = relu(segment_sum(vals * (xW)[src] by dst)))
on 8 Trainium2 NeuronCores.

Strategy (1D graph partitioning, per sharding hint):
- dst nodes are permuted into 8*BLOCKS blocks of 128, degree-balanced so
  every block has <= C*128 incoming edges; core m owns blocks
  [m*BLOCKS, (m+1)*BLOCKS) and produces those output rows.
- Every core computes the full dense transform h = x @ W (replicated; avoids
  any cross-core communication) into its DRAM, via float32r matmuls.
- Per dst block: one indirect DMA gathers the C*128 source rows h[src] into
  SBUF; a value-scaled one-hot matrix P (built on DVE from iota/is_equal) is
  matmul'd against the messages, accumulating all chunks into one PSUM tile:
  psum[d, f] = sum_e val[e] * [dst_e == d] * h[src_e, f]; relu + store.
- Padding edges carry val = 0 so they contribute nothing.
"""
import math
from contextlib import ExitStack

import numpy as np

import concourse.bass as bass
import concourse.mybir as mybir
import concourse.tile as tile
from concourse.bass_utils import run_bass_kernel_spmd
from concourse.vector_clock import ScopedClock

# --- workaround: this walrus build rejects >1 sync wait per instruction
# ("Too many sync wait commands"). Tile's kernel-tail drain carries one wait
# per active sem lane; give it the same NOP-splitting treatment as everything
# else via a post-schedule legalization pass over all basic blocks. ---
_MAX_WAITS = 1


def _patched_drain_and_barrier(self, tick_clock, wait_clock):
    drain_inst = self.nc.sync.drain()
    wait_clock.add_sem_waits(
        drain_inst.ins, ScopedClock({None: tick_clock.global_clock})
    )
    self.nc.all_engine_barrier()
    popped = self.nc._tile_sem_poison_stack.pop()
    assert popped is self._sem_poison
    self.nc.clear_and_free_semaphores(list(self.sems.allocated().values()))
    self.nc.all_engine_barrier()


tile.TileContext._drain_and_barrier = _patched_drain_and_barrier


def _legalize_sync_waits(nc):
    """Split instructions carrying >_MAX_WAITS sem waits: excess waits move to
    same-engine NOPs inserted immediately before the instruction."""
    n_split = 0
    for f in nc.m.functions:
        for bb in f.blocks:
            out = []
            changed = False
            for ins in bb.instructions:
                si = ins.sync_info
                waits = list(si.on_wait) if si and si.on_wait else []
                if len(waits) > _MAX_WAITS:
                    changed = True
                    n_split += 1
                    for i in range(_MAX_WAITS, len(waits), _MAX_WAITS):
                        nop = mybir.InstNoOp(
                            name=nc.get_next_instruction_name(), ins=[], outs=[]
                        )
                        nop.engine = ins.engine
                        nop.sync_info = mybir.SyncInfo(
                            on_wait=waits[i : i + _MAX_WAITS], on_update=[]
                        )
                        nc.register_instruction(nop, overwrite=True)
                        out.append(nop)
                    si.on_wait = waits[:_MAX_WAITS]
                out.append(ins)
            if changed:
                bb.instructions = out
    return n_split

N_CORES = 8
P = 128


def build_nc(n_pad, d_in, d_out, blocks, C, strip_blocks=8, phase_barrier=False, debug_h=False):
    """One SPMD program: out = relu((A @ x) @ W), gathering x rows directly.
    Gathers depend only on the input x, so the Pool-engine gather chain (the
    bottleneck) starts at t=0 instead of waiting for a dense h = xW pass.
    blocks: dst blocks per core. C: chunks (of 128 edges) per block."""
    f32 = mybir.dt.float32
    f32r = mybir.dt.float32r
    i32 = mybir.dt.int32
    KD = d_in // P

    nc = bass.Bass(num_swdge_queues=4, dynamic_dma_scratch_size=32768)
    xp = nc.declare_dram_parameter("x", [n_pad, d_in], f32r, isOutput=False)
    Wp = nc.declare_dram_parameter("W", [d_in, d_out], f32r, isOutput=False)
    srcp = nc.declare_dram_parameter("src", [P, blocks * C], i32, isOutput=False)
    dstp = nc.declare_dram_parameter("dstv", [P, blocks * C], f32, isOutput=False)
    valp = nc.declare_dram_parameter("valv", [P, blocks * C], f32, isOutput=False)
    idp = nc.declare_dram_parameter("ident", [P, P], f32r, isOutput=False)
    outp = nc.declare_dram_parameter("out", [blocks * P, d_out], f32, isOutput=True)

    with tile.TileContext(nc) as tc:
        with ExitStack() as ctx:
            wpool = ctx.enter_context(tc.tile_pool(name="w", bufs=1))
            epool = ctx.enter_context(tc.tile_pool(name="edges", bufs=1))
            mpool = ctx.enter_context(tc.tile_pool(name="msgs", bufs=2))
            ppool = ctx.enter_context(tc.tile_pool(name="onehot", bufs=2))
            psg = ctx.enter_context(tc.tile_pool(name="psg", bufs=2, space="PSUM"))
            gpool = ctx.enter_context(tc.tile_pool(name="gsb", bufs=2))
            pst = ctx.enter_context(tc.tile_pool(name="pst", bufs=2, space="PSUM"))
            tpool = ctx.enter_context(tc.tile_pool(name="gT", bufs=2))
            pso = ctx.enter_context(tc.tile_pool(name="pso", bufs=2, space="PSUM"))
            opool = ctx.enter_context(tc.tile_pool(name="osb", bufs=2))

            # --- constants / per-core edge data, loaded once ---
            w_t = wpool.tile([P, KD * d_out], f32r)
            for k in range(KD):
                nc.sync.dma_start(
                    w_t[:, k * d_out : (k + 1) * d_out], Wp[k * P : (k + 1) * P, :]
                )
            id_t = wpool.tile([P, P], f32r)
            nc.sync.dma_start(id_t[:], idp[:])
            src_t = epool.tile([P, blocks * C], i32)
            dst_t = epool.tile([P, blocks * C], f32)
            val_t = epool.tile([P, blocks * C], f32)
            nc.sync.dma_start(src_t[:], srcp[:])
            nc.sync.dma_start(dst_t[:], dstp[:])
            nc.sync.dma_start(val_t[:], valp[:])
            iota_t = epool.tile([P, C * P], f32)
            nc.gpsimd.iota(
                iota_t[:],
                pattern=[[0, C], [1, P]],
                base=0,
                channel_multiplier=0,
                allow_small_or_imprecise_dtypes=True,
            )

            # --- per dst block: gather x[src], one-hot scatter matmul to
            # g = A_b @ x, PE-transpose g, then out_b = relu(g @ W) ---
            for b in range(blocks):
                msgs = mpool.tile([P, C * d_in], f32r, tag="msgs")
                # HW indirect DMA honors one offset per partition per
                # instruction -> one gather of 128 rows per chunk.
                for c in range(C):
                    nc.gpsimd.indirect_dma_start(
                        out=msgs[:, c * d_in : (c + 1) * d_in],
                        out_offset=None,
                        in_=xp[:],
                        in_offset=bass.IndirectOffsetOnAxis(
                            ap=src_t[:, b * C + c : b * C + c + 1], axis=0
                        ),
                    )
                pt3 = ppool.tile([P, C * P], f32r, tag="P")
                iota3 = bass.AP(
                    iota_t[:].tensor, iota_t[:].offset,
                    [iota_t[:].ap[0], [P, C], [1, P]],
                )
                p3 = bass.AP(
                    pt3[:].tensor, pt3[:].offset, [pt3[:].ap[0], [P, C], [1, P]]
                )
                dstb = dst_t[:, b * C : (b + 1) * C]
                valb = val_t[:, b * C : (b + 1) * C]
                dst_bc = bass.AP(dstb.tensor, dstb.offset, dstb.ap + [[0, P]])
                val_bc = bass.AP(valb.tensor, valb.offset, valb.ap + [[0, P]])
                nc.vector.tensor_tensor(
                    out=p3, in0=iota3, in1=dst_bc, op=mybir.AluOpType.is_equal
                )
                nc.vector.tensor_tensor(
                    out=p3, in0=p3, in1=val_bc, op=mybir.AluOpType.mult
                )
                # g[dst-slot, k] = sum_e val*x[src_e, k]  -> [128, d_in] PSUM
                g_ps = psg.tile([P, d_in], f32)
                for c in range(C):
                    nc.tensor.matmul(
                        g_ps[:],
                        lhsT=pt3[:, c * P : (c + 1) * P],
                        rhs=msgs[:, c * d_in : (c + 1) * d_in],
                        start=(c == 0),
                        stop=(c == C - 1),
                    )
                g_sb = gpool.tile([P, d_in], f32r, tag="g")
                nc.scalar.activation(
                    g_sb[:], g_ps[:], mybir.ActivationFunctionType.Identity
                )
                # transpose g via identity matmuls, then out_b = relu(gT^T W)
                gT = tpool.tile([P, KD * P], f32r, tag="gT")
                for k in range(KD):
                    t_ps = pst.tile([P, P], f32)
                    nc.tensor.matmul(
                        t_ps[:],
                        lhsT=g_sb[:, k * P : (k + 1) * P],
                        rhs=id_t[:],
                        start=True,
                        stop=True,
                    )
                    nc.vector.tensor_copy(gT[:, k * P : (k + 1) * P], t_ps[:])
                o_ps = pso.tile([P, d_out], f32)
                for k in range(KD):
                    nc.tensor.matmul(
                        o_ps[:],
                        lhsT=gT[:, k * P : (k + 1) * P],
                        rhs=w_t[:, k * d_out : (k + 1) * d_out],
                        start=(k == 0),
                        stop=(k == KD - 1),
                    )
                ot = opool.tile([P, d_out], f32)
                nc.scalar.activation(ot[:], o_ps[:], mybir.ActivationFunctionType.Relu)
                nc.sync.dma_start(outp[b * P : (b + 1) * P, :], ot[:])
    # round-robin indirect gathers across the 4 SWDGE queues (ring-reclaim
    # stalls on a single ring otherwise serialize behind DMA completion)
    qi = 0
    for f in nc.m.functions:
        for bb in f.blocks:
            for ins in bb.instructions:
                if isinstance(ins, mybir.InstDMACopy) and ins.queue == "qPoolDynamic":
                    if qi % 4:
                        ins.queue = f"qPoolDynamic{qi % 4}"
                    qi += 1
    _legalize_sync_waits(nc)
    return nc


def _pack_edges(edge_src, edge_dst, edge_vals, n_nodes, blocks):
    """Permute dst nodes into degree-balanced blocks of 128; pack edges into
    [P, blocks_total*C] per-core arrays (chunk-major columns per block)."""
    import heapq

    total_blocks = N_CORES * blocks
    deg = np.bincount(edge_dst, minlength=n_nodes).astype(np.int64)
    order = np.argsort(-deg, kind="stable")
    heap = [(0, b) for b in range(total_blocks)]
    heapq.heapify(heap)
    counts = np.zeros(total_blocks, np.int32)
    loads = np.zeros(total_blocks, np.int64)
    block_of = np.empty(n_nodes, np.int32)
    slot_of = np.empty(n_nodes, np.int32)
    for node in order:
        while True:
            load, b = heapq.heappop(heap)
            if counts[b] < P:
                break
        block_of[node] = b
        slot_of[node] = counts[b]
        counts[b] += 1
        loads[b] = load + deg[node]
        if counts[b] < P:
            heapq.heappush(heap, (loads[b], b))
    C = max(1, math.ceil(loads.max() / P))

    eb = block_of[edge_dst]
    eorder = np.argsort(eb, kind="stable")
    eb_sorted = eb[eorder]
    bsizes = np.bincount(eb_sorted, minlength=total_blocks)
    boffs = np.concatenate([[0], np.cumsum(bsizes)[:-1]])
    pos = np.arange(len(edge_src)) - boffs[eb_sorted]

    src_a = np.zeros((total_blocks, C, P), np.int32)
    dst_a = np.zeros((total_blocks, C, P), np.float32)
    val_a = np.zeros((total_blocks, C, P), np.float32)
    ch, lane = pos >> 7, pos & 127
    src_a[eb_sorted, ch, lane] = edge_src[eorder]
    dst_a[eb_sorted, ch, lane] = slot_of[edge_dst[eorder]]
    val_a[eb_sorted, ch, lane] = edge_vals[eorder]

    def per_core(a):
        return [
            np.ascontiguousarray(
                a[m * blocks : (m + 1) * blocks].transpose(2, 0, 1).reshape(P, -1)
            )
            for m in range(N_CORES)
        ]

    return per_core(src_a), per_core(dst_a), per_core(val_a), block_of, slot_of, C


def _run(x, W, edge_vals, edge_src, edge_dst, blocks=None, trace=False, phase_barrier=False, trace_cores=None):
    n_nodes, d_in = x.shape
    d_out = W.shape[1]
    if blocks is None:
        blocks = math.ceil(n_nodes / (N_CORES * P))
    n_pad = math.ceil(n_nodes / P) * P
    src_c, dst_c, val_c, block_of, slot_of, C = _pack_edges(
        edge_src, edge_dst, edge_vals, n_nodes, blocks
    )
    xp = np.zeros((n_pad, d_in), np.float32)
    xp[:n_nodes] = x
    ident = np.eye(P, dtype=np.float32)

    nc = build_nc(n_pad, d_in, d_out, blocks, C, phase_barrier=phase_barrier)
    in_maps = [
        {
            "x": xp,
            "W": W,
            "src": src_c[m],
            "dstv": dst_c[m],
            "valv": val_c[m],
            "ident": ident,
        }
        for m in range(N_CORES)
    ]
    res = run_bass_kernel_spmd(
        nc, in_maps, list(range(N_CORES)), trace=trace, trace_cores=trace_cores
    )
    stacked = np.concatenate([res.results[m]["out"] for m in range(N_CORES)], axis=0)
    npc = blocks * P
    gidx = (block_of // blocks) * npc + (block_of % blocks) * P + slot_of
    out = stacked[gidx]
    return out, res


def kernel(x, W, edge_vals, edge_src, edge_dst):
    x = np.asarray(x, np.float32)
    W = np.asarray(W, np.float32)
    edge_vals = np.asarray(edge_vals, np.float32)
    edge_src = np.asarray(edge_src).astype(np.int64)
    edge_dst = np.asarray(edge_dst).astype(np.int64)
    out, _ = _run(x, W, edge_vals, edge_src, edge_dst)
    return out.astype(np.float32)

